# revision 20
# baseline (speedup 1.0000x reference)
"""Trainium2 Bass kernel for nn_MultiHeadClassifier.

  logits[b, c] = sum_{(g,l): label_ids[g,l]==c} group_probs[b,g] *
                 (features[b] @ W[g,l] + b[g,l])

Data-parallel over batch (8 cores, 4096 rows each). Per core:
  * Host prep: sort the G*L=1024 head outputs by target class (no
    padding -> exactly NCH=8 chunks of 128). Classes straddling a chunk
    boundary are handled by 1-wide accumulate fixup matmuls.
  * GEMM1 (PE, bf16): pg[gl, b] = Wsorted^T.T @ X^T per chunk/b-tile.
  * M-matmul (PE, bf16): pm[gl, b] = E_j.T @ pT (group-prob gather as a
    0/1 matmul).
  * DVE scalar_tensor_tensor: wtj = (pg + bias) * pm, PSUM-direct.
  * Scatter (PE, bf16): pl[b, lo_j:hi_j] = wtjT_j.T @ S_j with S_j a
    0/1 band matrix; disjoint bands + fixup columns for straddlers.
  * Drain pl (PSUM f32) to SBUF split across ACT/DVE/Pool, DMA out.
"""
import os
import sys
import numpy as np
import ml_dtypes

for _p in ("/opt/trn_rl_repo",):
    if _p not in sys.path:
        sys.path.append(_p)

import concourse.bass as bass  # noqa: E402
import concourse.tile as tile  # noqa: E402
from concourse import bacc, mybir, bass_utils  # noqa: E402
from contextlib import ExitStack  # noqa: E402

F32 = mybir.dt.float32
BF16 = mybir.dt.bfloat16

B, F, G, L, C = 32768, 512, 16, 64, 1000
NCORE = 8
BC = B // NCORE          # 4096 batch rows per core
NT = BC // 512           # 8 b-tiles of 512
KF = F // 128            # 4 feature chunks
GL = G * L               # 1024 heads
NCH = GL // 128          # 8 head chunks, no padding

LAST_EXEC_NS = None


def _ensure_ntff_hook():
    """Some images ship an `antenv` without the optional `axon_hooks`
    submodule; bass_utils then crashes on import when tracing. Provide
    the module and register the ctypes NTFF hook trn_boot would have."""
    try:
        from antenv import axon_hooks  # noqa: F401
        return
    except ImportError:
        pass
    import types
    import antenv
    mod = types.ModuleType("antenv.axon_hooks")
    _hook = [None]
    mod.set_axon_ntff_profile_hook = lambda h: _hook.__setitem__(0, h)
    mod.get_axon_ntff_profile_hook = lambda: _hook[0]
    sys.modules["antenv.axon_hooks"] = mod
    antenv.axon_hooks = mod
    try:
        from trn_agent_boot.trn_boot import _ntff_profile_via_ctypes
        h = _ntff_profile_via_ctypes("/opt/axon/libaxon_pjrt.so")
        if h is not None:
            mod.set_axon_ntff_profile_hook(h)
    except Exception:
        pass


def _host_prep(W, b, label_ids):
    lab = np.asarray(label_ids).reshape(-1).astype(np.int64)
    order = np.argsort(lab, kind="stable")
    lab_s = lab[order]                      # ascending classes, len 1024

    # bands + straddle fixups
    bands = []
    fixups = []                             # (chunk_j, cstar, fix_col)
    lo = 0
    for j in range(NCH):
        if j < NCH - 1:
            c_end = int(lab_s[128 * (j + 1) - 1])
            c_next = int(lab_s[128 * (j + 1)])
            if c_end == c_next:
                hi = c_end + 1
                fixups.append((j + 1, c_end, len(fixups)))
            else:
                hi = c_next
        else:
            hi = C
        bands.append((lo, hi))
        lo = hi

    straddle_tail = {(j, c) for (j, c, _) in fixups}
    NFIX = max(len(fixups), 1)
    S_cat = np.zeros((128, C), dtype=ml_dtypes.bfloat16)
    S_fix = np.zeros((128, NFIX), dtype=ml_dtypes.bfloat16)
    fix_of = {(j, c): col for (j, c, col) in fixups}
    for p in range(GL):
        j, r = p // 128, p % 128
        c = int(lab_s[p])
        if (j, c) in straddle_tail:
            S_fix[r, fix_of[(j, c)]] = 1.0
        else:
            S_cat[r, c] = 1.0

    Wflat = np.asarray(W).reshape(GL, F).astype(np.float32)
    bflat = np.asarray(b).reshape(GL).astype(np.float32)
    Wsorted = Wflat[order]                  # [1024, F]
    # wt tiles: [KF, 2, 128, 512] -> contiguous [128,512] blocks
    WT2 = np.ascontiguousarray(
        Wsorted.T.reshape(KF, 128, GL).reshape(KF, 128, 2, 512)
        .transpose(0, 2, 1, 3)).astype(ml_dtypes.bfloat16)
    biasT = np.zeros((128, NCH), dtype=np.float32)
    E = np.zeros((16, GL), dtype=ml_dtypes.bfloat16)
    for p, gl in enumerate(order):
        biasT[p % 128, p // 128] = bflat[gl]
        E[gl // L, p] = 1.0
    return dict(bands=bands, fixups=fixups, NFIX=NFIX, S_cat=S_cat,
                S_fix=S_fix, WT2=WT2, biasT=biasT, E=E)


def _band_segments(lo, hi, fix_cols):
    """Split [lo, hi) at 512-col (PSUM bank) boundaries; a segment whose
    range contains a fixup target column keeps its accumulation open."""
    segs = []
    while lo < hi:
        nxt = min(hi, (lo // 512 + 1) * 512)
        stop = not any(lo <= c < nxt for c in fix_cols)
        segs.append((lo, nxt, stop))
        lo = nxt
    return segs


def _build_program(bands, fixups, NFIX):
    nc = bacc.Bacc("TRN2", target_bir_lowering=False, debug=False,
                   num_devices=NCORE)
    xt_d = nc.dram_tensor("xt", [KF * NT * 128, 512], BF16,
                          kind="ExternalInput").ap()
    pt_d = nc.dram_tensor("pt", [16, BC], BF16, kind="ExternalInput").ap()
    wt_d = nc.dram_tensor("wt", [KF * 2 * 128, 512], BF16,
                          kind="ExternalInput").ap()
    e_d = nc.dram_tensor("e", [16, GL], BF16, kind="ExternalInput").ap()
    bt_d = nc.dram_tensor("bt", [128, NCH], F32, kind="ExternalInput").ap()
    s_d = nc.dram_tensor("s", [128, C], BF16, kind="ExternalInput").ap()
    sf_d = nc.dram_tensor("sf", [128, NFIX], BF16, kind="ExternalInput").ap()
    out_d = nc.dram_tensor("logits", [BC, C], F32, kind="ExternalOutput").ap()

    # fixup targets per chunk: chunk j -> [(cstar, col)]
    fix_by_chunk = {}
    for (j, c, col) in fixups:
        fix_by_chunk.setdefault(j, []).append((c, col))
    # per chunk: columns that a LATER fixup will accumulate into
    fix_cols_of_band = {}
    for (j, c, col) in fixups:
        fix_cols_of_band.setdefault(j - 1, []).append(c)

    with tile.TileContext(nc) as tc, ExitStack() as ctx:
        const = ctx.enter_context(tc.tile_pool(name="const", bufs=1))
        psG = ctx.enter_context(tc.tile_pool(name="psG", bufs=3, space="PSUM"))
        psM = ctx.enter_context(tc.tile_pool(name="psM", bufs=3, space="PSUM"))
        psLA = ctx.enter_context(tc.tile_pool(name="psLA", bufs=1, space="PSUM"))
        psLB = ctx.enter_context(tc.tile_pool(name="psLB", bufs=1, space="PSUM"))
        sbW = ctx.enter_context(tc.tile_pool(name="sbW", bufs=18))
        sbP = ctx.enter_context(tc.tile_pool(name="sbP", bufs=4))
        sbO = ctx.enter_context(tc.tile_pool(name="sbO", bufs=6))

        # small consts on the gpsimd (software) queue; pts first (pm j=0 dep)
        pts = const.tile([16, BC], BF16, name="pts", tag="pts")
        nc.gpsimd.dma_start(pts[:], pt_d[:])
        es = const.tile([16, GL], BF16, name="es", tag="es")
        nc.gpsimd.dma_start(es[:], e_d[:])
        bts = const.tile([128, NCH], F32, name="bts", tag="bts")
        nc.gpsimd.dma_start(bts[:], bt_d[:])
        ss = const.tile([128, C], BF16, name="ss", tag="ss")
        nc.gpsimd.dma_start(ss[:], s_d[:])
        sfs = const.tile([128, NFIX], BF16, name="sfs", tag="sfs")
        nc.gpsimd.dma_start(sfs[:], sf_d[:])

        # weights + activations on the sync hw queue; first-gemm deps first
        wts = [[None, None] for _ in range(KF)]
        xts = [[None] * NT for _ in range(KF)]

        def load_wt(k, h):
            t_ = const.tile([128, 512], BF16, name=f"w{k}_{h}", tag=f"w{k}_{h}")
            nc.sync.dma_start(t_[:], wt_d[(k * 2 + h) * 128:
                                          (k * 2 + h + 1) * 128, :])
            wts[k][h] = t_

        def load_x(k, t, eng):
            t_ = const.tile([128, 512], BF16, name=f"x{k}_{t}", tag=f"x{k}_{t}")
            eng.dma_start(t_[:], xt_d[(k * NT + t) * 128:
                                      (k * NT + t + 1) * 128, :])
            xts[k][t] = t_

        # first-tile x on the scalar queue so its transfers run in
        # parallel with the wt loads on sync during startup
        for k in range(KF):
            load_wt(k, 0)
            load_x(k, 0, nc.scalar)
        for k in range(KF):
            load_wt(k, 1)
        for t in range(1, NT):
            for k in range(KF):
                load_x(k, t, nc.sync)

        def wt_slice(k, j):
            """lhsT [128, 128] for feature-chunk k, head-chunk j."""
            h, off = divmod(j * 128, 512)
            return wts[k][h][:, off:off + 128]

        # tiles: 7 full 512-row tiles + 2 half tiles at the end, so the
        # final scatter tail only covers 256 rows
        TILES = [(ti * 512, 512) for ti in range(NT - 1)]
        TILES += [(3584, 256), (3840, 256)]

        all_wtjs = {}

        def gemm_chunk(ti, j):
            row0, w = TILES[ti]
            tx, off = divmod(row0, 512)
            pg = psG.tile([128, 512], F32, name="pg", tag="pg")
            pm = psM.tile([128, 512], F32, name="pm", tag="pm")
            for k in range(KF):
                nc.tensor.matmul(pg[:, :w], wt_slice(k, j),
                                 xts[k][tx][:, off:off + w],
                                 start=(k == 0), stop=(k == KF - 1))
                if k == 1:
                    # interleave the prob-gather matmul inside the open pg
                    # accumulation group: LDWEIGHTS after an in-group matmul
                    # overlaps its streaming; after a group close it stalls
                    # until the array drains (~300ns per transition)
                    nc.tensor.matmul(pm[:, :w], es[:, bass.ts(j, 128)],
                                     pts[:, row0:row0 + w],
                                     start=True, stop=True)
            # hw limit: only one PSUM operand per DVE op -> drain pg on ACT
            gb = sbP.tile([128, 512], BF16, name="gb", tag="gb")
            nc.scalar.activation(gb[:, :w], pg[:, :w],
                                 mybir.ActivationFunctionType.Identity,
                                 bias=bts[:, j:j + 1], scale=1.0)
            wtj = sbW.tile([128, 512], BF16, name="wtj", tag="wtj")
            nc.vector.tensor_mul(wtj[:, :w], gb[:, :w], pm[:, :w])
            all_wtjs.setdefault(ti, []).append(wtj)

        pending_drain = []

        def scatter_mm(ti, bs_i):
            wtjs = all_wtjs[ti]
            bsl = bass.ts(bs_i, 128)
            # two single-bank tiles so each drains/recycles independently
            plA = psLA.tile([128, 512], F32, name="plA", tag="plA")
            plB = psLB.tile([128, 512], F32, name="plB", tag="plB")

            def pslice(n0, n1):
                return plA[:, n0:n1] if n1 <= 512 else plB[:, n0 - 512:n1 - 512]

            for j, (lo, hi) in enumerate(bands):
                for (n0, n1, stop) in _band_segments(
                        lo, hi, fix_cols_of_band.get(j, ())):
                    nc.tensor.matmul(pslice(n0, n1), wtjs[j][:, bsl],
                                     ss[:, n0:n1], start=True, stop=stop)
                # a later band's start=True re-marks the whole 2KB zero
                # region pending-zero, which would wipe the straddle column
                # -> the fixup accumulate must run before band j+1 starts
                for (cstar, col) in fix_by_chunk.get(j + 1, ()):
                    nc.tensor.matmul(pslice(cstar, cstar + 1),
                                     wtjs[j + 1][:, bsl],
                                     sfs[:, col:col + 1],
                                     start=False, stop=True)
            pending_drain.append((ti, bs_i, plA, plB))

        def scatter_drain():
            while pending_drain:
                ti, bs_i, plA, plB = pending_drain.pop(0)
                row0 = TILES[ti][0] + bs_i * 128
                ob = sbO.tile([128, C], F32, name="ob", tag="ob")
                # gpsimd cannot read PSUM; split the drain ACT/DVE
                nc.scalar.activation(ob[:, :512], plA[:],
                                     mybir.ActivationFunctionType.Identity,
                                     bias=0.0, scale=1.0)
                nc.vector.tensor_copy(ob[:, 512:C], plB[:, 0:C - 512])
                # alternate output queues: keep half off the input (sync)
                # queue and half off the scalar engine's stream
                eng = nc.scalar if bs_i % 2 == 0 else nc.sync
                eng.dma_start(out_d[row0:row0 + 128, :], ob[:])

        # software pipeline: scatter(t-1) subtiles interleaved into gemm(t);
        # drains emitted one chunk later so gb/mult stay ahead in the
        # ACT/DVE queues
        NTI = len(TILES)
        for ti in range(NTI):
            nsub_prev = TILES[ti - 1][1] // 128 if ti > 0 else 0
            slots = {3: 0, 7: 1} if nsub_prev == 2 else \
                    {1: 0, 3: 1, 5: 2, 7: 3}
            for j in range(NCH):
                gemm_chunk(ti, j)
                scatter_drain()
                if ti > 0 and j in slots:
                    scatter_mm(ti - 1, slots[j])
            if ti > 0:
                all_wtjs.pop(ti - 1)
        for bs_i in range(TILES[-1][1] // 128):
            scatter_mm(NTI - 1, bs_i)
            scatter_drain()
        all_wtjs.pop(NTI - 1)
    nc.finalize()
    return nc


def kernel(features, group_probs, W, b, label_ids):
    global LAST_EXEC_NS
    features = np.asarray(features, dtype=np.float32)
    group_probs = np.asarray(group_probs, dtype=np.float32)
    prep = _host_prep(W, b, label_ids)
    nc = _build_program(prep["bands"], prep["fixups"], prep["NFIX"])

    Xb = features.astype(ml_dtypes.bfloat16)
    PT = np.ascontiguousarray(group_probs.T.astype(ml_dtypes.bfloat16))
    WT2 = np.ascontiguousarray(prep["WT2"].reshape(KF * 2 * 128, 512))
    in_maps = []
    for c in range(NCORE):
        Xc = Xb[c * BC:(c + 1) * BC]                      # [4096, 512]
        # [KF, NT, 128, 512]: tile (k, t) = Xc[t*512:(t+1)*512,
        #                                      k*128:(k+1)*128].T
        XT2 = np.ascontiguousarray(
            Xc.reshape(NT, 512, KF, 128).transpose(2, 0, 3, 1))
        in_maps.append({
            "xt": XT2.reshape(KF * NT * 128, 512),
            "pt": np.ascontiguousarray(PT[:, c * BC:(c + 1) * BC]),
            "wt": WT2,
            "e": prep["E"],
            "bt": prep["biasT"],
            "s": prep["S_cat"],
            "sf": prep["S_fix"],
        })

    trace = bool(os.environ.get("BASS_TRACE"))
    if trace:
        bass_utils.upload_artifacts = lambda d: "local://skipped"
        _ensure_ntff_hook()
    try:
        res = bass_utils.run_bass_kernel_spmd(nc, in_maps,
                                              core_ids=list(range(NCORE)))
    except Exception:
        # transient NRT device errors have been observed; one retry
        res = bass_utils.run_bass_kernel_spmd(nc, in_maps,
                                              core_ids=list(range(NCORE)))
    if trace:
        LAST_EXEC_NS = res.exec_time_ns
        if res.exec_time_ns is not None:
            print(f"HW exec time: {res.exec_time_ns} ns")

    out = np.concatenate([res.results[c]["logits"] for c in range(NCORE)],
                         axis=0)
    return np.ascontiguousarray(out.astype(np.float32))


# revision 24
# speedup vs baseline: 1.0243x; 1.0243x over previous
"""Trainium2 Bass kernel for nn_MultiHeadClassifier.

  logits[b, c] = sum_{(g,l): label_ids[g,l]==c} group_probs[b,g] *
                 (features[b] @ W[g,l] + b[g,l])

Data-parallel over batch (8 cores, 4096 rows each). Per core:
  * Host prep: sort the G*L=1024 head outputs by target class (no
    padding -> exactly NCH=8 chunks of 128). Classes straddling a chunk
    boundary are handled by 1-wide accumulate fixup matmuls.
  * GEMM1 (PE, bf16): pg[gl, b] = Wsorted^T.T @ X^T per chunk/b-tile.
  * M-matmul (PE, bf16): pm[gl, b] = E_j.T @ pT (group-prob gather as a
    0/1 matmul).
  * ACT: gb = bf16(pg + bias); DVE: wtj = gb * pm (one PSUM operand max).
  * Scatter (PE, bf16): pl[b, lo_j:hi_j] = wtjT_j.T @ S_j with S_j a
    0/1 band matrix; disjoint bands + fixup columns for straddlers.
  * Drain pl (PSUM f32) to SBUF split across ACT/DVE, DMA out.
"""
import os
import sys
import numpy as np
import ml_dtypes

for _p in ("/opt/trn_rl_repo",):
    if _p not in sys.path:
        sys.path.append(_p)

import concourse.bass as bass  # noqa: E402
import concourse.tile as tile  # noqa: E402
from concourse import bacc, mybir, bass_utils  # noqa: E402
from contextlib import ExitStack  # noqa: E402

F32 = mybir.dt.float32
BF16 = mybir.dt.bfloat16

B, F, G, L, C = 32768, 512, 16, 64, 1000
NCORE = 8
BC = B // NCORE          # 4096 batch rows per core
NT = BC // 512           # 8 b-tiles of 512
KF = F // 128            # 4 feature chunks
GL = G * L               # 1024 heads
NCH = GL // 128          # 8 head chunks, no padding

LAST_EXEC_NS = None


def _ensure_ntff_hook():
    """Some images ship an `antenv` without the optional `axon_hooks`
    submodule; bass_utils then crashes on import when tracing. Provide
    the module and register the ctypes NTFF hook trn_boot would have."""
    try:
        from antenv import axon_hooks  # noqa: F401
        return
    except ImportError:
        pass
    import types
    import antenv
    mod = types.ModuleType("antenv.axon_hooks")
    _hook = [None]
    mod.set_axon_ntff_profile_hook = lambda h: _hook.__setitem__(0, h)
    mod.get_axon_ntff_profile_hook = lambda: _hook[0]
    sys.modules["antenv.axon_hooks"] = mod
    antenv.axon_hooks = mod
    try:
        from trn_agent_boot.trn_boot import _ntff_profile_via_ctypes
        h = _ntff_profile_via_ctypes("/opt/axon/libaxon_pjrt.so")
        if h is not None:
            mod.set_axon_ntff_profile_hook(h)
    except Exception:
        pass


def _host_prep(W, b, label_ids):
    lab = np.asarray(label_ids).reshape(-1).astype(np.int64)
    order = np.argsort(lab, kind="stable")
    lab_s = lab[order]                      # ascending classes, len 1024

    # bands + straddle fixups
    bands = []
    fixups = []                             # (chunk_j, cstar, fix_col)
    lo = 0
    for j in range(NCH):
        if j < NCH - 1:
            c_end = int(lab_s[128 * (j + 1) - 1])
            c_next = int(lab_s[128 * (j + 1)])
            if c_end == c_next:
                hi = c_end + 1
                fixups.append((j + 1, c_end, len(fixups)))
            else:
                hi = c_next
        else:
            hi = C
        bands.append((lo, hi))
        lo = hi

    straddle_tail = {(j, c) for (j, c, _) in fixups}
    NFIX = max(len(fixups), 1)
    S_cat = np.zeros((128, C), dtype=ml_dtypes.bfloat16)
    S_fix = np.zeros((128, NFIX), dtype=ml_dtypes.bfloat16)
    fix_of = {(j, c): col for (j, c, col) in fixups}
    for p in range(GL):
        j, r = p // 128, p % 128
        c = int(lab_s[p])
        if (j, c) in straddle_tail:
            S_fix[r, fix_of[(j, c)]] = 1.0
        else:
            S_cat[r, c] = 1.0

    Wflat = np.asarray(W).reshape(GL, F).astype(np.float32)
    bflat = np.asarray(b).reshape(GL).astype(np.float32)
    Wsorted = Wflat[order]                  # [1024, F]
    # wt tiles: [KF, 2, 128, 512] -> contiguous [128,512] blocks
    WT2 = np.ascontiguousarray(
        Wsorted.T.reshape(KF, 128, GL).reshape(KF, 128, 2, 512)
        .transpose(0, 2, 1, 3)).astype(ml_dtypes.bfloat16)
    biasT = np.zeros((128, NCH), dtype=np.float32)
    E = np.zeros((16, GL), dtype=ml_dtypes.bfloat16)
    for p, gl in enumerate(order):
        biasT[p % 128, p // 128] = bflat[gl]
        E[gl // L, p] = 1.0
    return dict(bands=bands, fixups=fixups, NFIX=NFIX, S_cat=S_cat,
                S_fix=S_fix, WT2=WT2, biasT=biasT, E=E)


def _band_segments(lo, hi, fix_cols):
    """Split [lo, hi) at 512-col (PSUM bank) boundaries; a segment whose
    range contains a fixup target column keeps its accumulation open."""
    segs = []
    while lo < hi:
        nxt = min(hi, (lo // 512 + 1) * 512)
        stop = not any(lo <= c < nxt for c in fix_cols)
        segs.append((lo, nxt, stop))
        lo = nxt
    return segs


def _build_program(bands, fixups, NFIX):
    nc = bacc.Bacc("TRN2", target_bir_lowering=False, debug=False,
                   num_devices=NCORE)
    xt_d = nc.dram_tensor("xt", [KF * NT * 128, 512], BF16,
                          kind="ExternalInput").ap()
    pt_d = nc.dram_tensor("pt", [16, BC], BF16, kind="ExternalInput").ap()
    wt_d = nc.dram_tensor("wt", [KF * 2 * 128, 512], BF16,
                          kind="ExternalInput").ap()
    e_d = nc.dram_tensor("e", [16, GL], BF16, kind="ExternalInput").ap()
    bt_d = nc.dram_tensor("bt", [128, NCH], F32, kind="ExternalInput").ap()
    s_d = nc.dram_tensor("s", [128, C], BF16, kind="ExternalInput").ap()
    sf_d = nc.dram_tensor("sf", [128, NFIX], BF16, kind="ExternalInput").ap()
    out_d = nc.dram_tensor("logits", [BC, C], F32, kind="ExternalOutput").ap()

    # fixup targets per chunk: chunk j -> [(cstar, col)]
    fix_by_chunk = {}
    for (j, c, col) in fixups:
        fix_by_chunk.setdefault(j, []).append((c, col))
    # per chunk: columns that a LATER fixup will accumulate into
    fix_cols_of_band = {}
    for (j, c, col) in fixups:
        fix_cols_of_band.setdefault(j - 1, []).append(c)

    with tile.TileContext(nc) as tc, ExitStack() as ctx:
        const = ctx.enter_context(tc.tile_pool(name="const", bufs=1))
        psG = ctx.enter_context(tc.tile_pool(name="psG", bufs=2, space="PSUM"))
        psM = ctx.enter_context(tc.tile_pool(name="psM", bufs=3, space="PSUM"))
        psLA = ctx.enter_context(tc.tile_pool(name="psLA", bufs=2, space="PSUM"))
        psLB = ctx.enter_context(tc.tile_pool(name="psLB", bufs=1, space="PSUM"))
        sbW = ctx.enter_context(tc.tile_pool(name="sbW", bufs=18))
        sbP = ctx.enter_context(tc.tile_pool(name="sbP", bufs=4))
        sbO = ctx.enter_context(tc.tile_pool(name="sbO", bufs=6))

        # first-tile x slice k=2 leads the gpsimd (software) queue so all
        # first-gemm transfers run on three queues in parallel at startup
        x20 = const.tile([128, 512], BF16, name="x2_0", tag="x2_0")
        nc.gpsimd.dma_start(x20[:], xt_d[(2 * NT) * 128:(2 * NT + 1) * 128, :])
        # small consts follow; pts first (pm j=0 dep)
        pts = const.tile([16, BC], BF16, name="pts", tag="pts")
        nc.gpsimd.dma_start(pts[:], pt_d[:])
        es = const.tile([16, GL], BF16, name="es", tag="es")
        nc.gpsimd.dma_start(es[:], e_d[:])
        bts = const.tile([128, NCH], F32, name="bts", tag="bts")
        nc.gpsimd.dma_start(bts[:], bt_d[:])
        ss = const.tile([128, C], BF16, name="ss", tag="ss")
        nc.gpsimd.dma_start(ss[:], s_d[:])
        sfs = const.tile([128, NFIX], BF16, name="sfs", tag="sfs")
        nc.gpsimd.dma_start(sfs[:], sf_d[:])

        # weights + activations on the sync hw queue; first-gemm deps first
        wts = [[None, None] for _ in range(KF)]
        xts = [[None] * NT for _ in range(KF)]

        def load_wt(k, h):
            t_ = const.tile([128, 512], BF16, name=f"w{k}_{h}", tag=f"w{k}_{h}")
            nc.sync.dma_start(t_[:], wt_d[(k * 2 + h) * 128:
                                          (k * 2 + h + 1) * 128, :])
            wts[k][h] = t_

        def load_x(k, t, eng):
            t_ = const.tile([128, 512], BF16, name=f"x{k}_{t}", tag=f"x{k}_{t}")
            eng.dma_start(t_[:], xt_d[(k * NT + t) * 128:
                                      (k * NT + t + 1) * 128, :])
            xts[k][t] = t_

        # first-tile x on the scalar queue so its transfers run in
        # parallel with the wt loads on sync during startup (k=2 was
        # already issued on the gpsimd queue above)
        xts[2][0] = x20
        for k in range(KF):
            load_wt(k, 0)
            if k != 2:
                load_x(k, 0, nc.scalar)
        for k in range(KF):
            load_wt(k, 1)
        for t in range(1, NT):
            for k in range(KF):
                load_x(k, t, nc.sync)

        def wt_slice(k, j):
            """lhsT [128, 128] for feature-chunk k, head-chunk j."""
            h, off = divmod(j * 128, 512)
            return wts[k][h][:, off:off + 128]

        # tiles: 7 full 512-row tiles + 2 half tiles at the end, so the
        # final scatter tail only covers 256 rows
        TILES = [(ti * 512, 512) for ti in range(NT - 1)]
        TILES += [(3584, 256), (3840, 256)]

        all_wtjs = {}

        def gemm_chunk(ti, j):
            row0, w = TILES[ti]
            tx, off = divmod(row0, 512)
            pg = psG.tile([128, 512], F32, name="pg", tag="pg")
            pm = psM.tile([128, 512], F32, name="pm", tag="pm")
            for k in range(KF):
                nc.tensor.matmul(pg[:, :w], wt_slice(k, j),
                                 xts[k][tx][:, off:off + w],
                                 start=(k == 0), stop=(k == KF - 1))
                if k == 1:
                    # interleave the prob-gather matmul inside the open pg
                    # accumulation group: LDWEIGHTS after an in-group matmul
                    # overlaps its streaming; after a group close it stalls
                    # until the array drains (~300ns per transition)
                    nc.tensor.matmul(pm[:, :w], es[:, bass.ts(j, 128)],
                                     pts[:, row0:row0 + w],
                                     start=True, stop=True)
            # hw limit: only one PSUM operand per DVE op -> drain pg on ACT
            gb = sbP.tile([128, 512], BF16, name="gb", tag="gb")
            nc.scalar.activation(gb[:, :w], pg[:, :w],
                                 mybir.ActivationFunctionType.Identity,
                                 bias=bts[:, j:j + 1], scale=1.0)
            wtj = sbW.tile([128, 512], BF16, name="wtj", tag="wtj")
            nc.vector.tensor_mul(wtj[:, :w], gb[:, :w], pm[:, :w])
            all_wtjs.setdefault(ti, []).append(wtj)

        pending_drain = []

        def scatter_mm(ti, bs_i):
            wtjs = all_wtjs[ti]
            bsl = bass.ts(bs_i, 128)
            # two single-bank tiles so each drains/recycles independently
            plA = psLA.tile([128, 512], F32, name="plA", tag="plA")
            plB = psLB.tile([128, 512], F32, name="plB", tag="plB")

            def pslice(n0, n1):
                return plA[:, n0:n1] if n1 <= 512 else plB[:, n0 - 512:n1 - 512]

            for j, (lo, hi) in enumerate(bands):
                for (n0, n1, stop) in _band_segments(
                        lo, hi, fix_cols_of_band.get(j, ())):
                    nc.tensor.matmul(pslice(n0, n1), wtjs[j][:, bsl],
                                     ss[:, n0:n1], start=True, stop=stop)
                # a later band's start=True re-marks the whole 2KB zero
                # region pending-zero, which would wipe the straddle column
                # -> the fixup accumulate must run before band j+1 starts
                for (cstar, col) in fix_by_chunk.get(j + 1, ()):
                    nc.tensor.matmul(pslice(cstar, cstar + 1),
                                     wtjs[j + 1][:, bsl],
                                     sfs[:, col:col + 1],
                                     start=False, stop=True)
            pending_drain.append((ti, bs_i, plA, plB))

        def scatter_drain():
            while pending_drain:
                ti, bs_i, plA, plB = pending_drain.pop(0)
                row0 = TILES[ti][0] + bs_i * 128
                ob = sbO.tile([128, C], F32, name="ob", tag="ob")
                # gpsimd cannot read PSUM; split the drain ACT/DVE
                nc.scalar.activation(ob[:, :512], plA[:],
                                     mybir.ActivationFunctionType.Identity,
                                     bias=0.0, scale=1.0)
                nc.vector.tensor_copy(ob[:, 512:C], plB[:, 0:C - 512])
                # alternate output queues: keep half off the input (sync)
                # queue and half off the scalar engine's stream
                eng = nc.scalar if bs_i % 2 == 0 else nc.sync
                eng.dma_start(out_d[row0:row0 + 128, :], ob[:])

        # software pipeline: scatter(t-1) subtiles interleaved into gemm(t);
        # drains emitted one chunk later so gb/mult stay ahead in the
        # ACT/DVE queues
        NTI = len(TILES)
        for ti in range(NTI):
            nsub_prev = TILES[ti - 1][1] // 128 if ti > 0 else 0
            slots = {3: 0, 7: 1} if nsub_prev == 2 else \
                    {1: 0, 3: 1, 5: 2, 7: 3}
            for j in range(NCH):
                gemm_chunk(ti, j)
                scatter_drain()
                if ti > 0 and j in slots:
                    scatter_mm(ti - 1, slots[j])
            if ti > 0:
                all_wtjs.pop(ti - 1)
        for bs_i in range(TILES[-1][1] // 128):
            scatter_mm(NTI - 1, bs_i)
            scatter_drain()
        all_wtjs.pop(NTI - 1)
    nc.finalize()
    return nc


def kernel(features, group_probs, W, b, label_ids):
    global LAST_EXEC_NS
    features = np.asarray(features, dtype=np.float32)
    group_probs = np.asarray(group_probs, dtype=np.float32)
    prep = _host_prep(W, b, label_ids)
    nc = _build_program(prep["bands"], prep["fixups"], prep["NFIX"])

    Xb = features.astype(ml_dtypes.bfloat16)
    PT = np.ascontiguousarray(group_probs.T.astype(ml_dtypes.bfloat16))
    WT2 = np.ascontiguousarray(prep["WT2"].reshape(KF * 2 * 128, 512))
    in_maps = []
    for c in range(NCORE):
        Xc = Xb[c * BC:(c + 1) * BC]                      # [4096, 512]
        # [KF, NT, 128, 512]: tile (k, t) = Xc[t*512:(t+1)*512,
        #                                      k*128:(k+1)*128].T
        XT2 = np.ascontiguousarray(
            Xc.reshape(NT, 512, KF, 128).transpose(2, 0, 3, 1))
        in_maps.append({
            "xt": XT2.reshape(KF * NT * 128, 512),
            "pt": np.ascontiguousarray(PT[:, c * BC:(c + 1) * BC]),
            "wt": WT2,
            "e": prep["E"],
            "bt": prep["biasT"],
            "s": prep["S_cat"],
            "sf": prep["S_fix"],
        })

    trace = bool(os.environ.get("BASS_TRACE"))
    if trace:
        bass_utils.upload_artifacts = lambda d: "local://skipped"
        _ensure_ntff_hook()
    try:
        res = bass_utils.run_bass_kernel_spmd(nc, in_maps,
                                              core_ids=list(range(NCORE)))
    except Exception:
        # transient NRT device errors have been observed; one retry
        res = bass_utils.run_bass_kernel_spmd(nc, in_maps,
                                              core_ids=list(range(NCORE)))
    if trace:
        LAST_EXEC_NS = res.exec_time_ns
        if res.exec_time_ns is not None:
            print(f"HW exec time: {res.exec_time_ns} ns")

    out = np.concatenate([res.results[c]["logits"] for c in range(NCORE)],
                         axis=0)
    return np.ascontiguousarray(out.astype(np.float32))


# revision 26
# speedup vs baseline: 1.0449x; 1.0201x over previous
"""Trainium2 Bass kernel for nn_MultiHeadClassifier.

  logits[b, c] = sum_{(g,l): label_ids[g,l]==c} group_probs[b,g] *
                 (features[b] @ W[g,l] + b[g,l])

Data-parallel over batch (8 cores, 4096 rows each). Per core:
  * Host prep: sort the G*L=1024 head outputs by target class (no
    padding -> exactly NCH=8 chunks of 128). Classes straddling a chunk
    boundary are handled by 1-wide accumulate fixup matmuls.
  * GEMM1 (PE, bf16): pg[gl, b] = Wsorted^T.T @ X^T per chunk/b-tile.
  * M-matmul (PE, bf16): pm[gl, b] = E_j.T @ pT (group-prob gather as a
    0/1 matmul).
  * ACT: gb = bf16(pg + bias); DVE: wtj = gb * pm (one PSUM operand max).
  * Scatter (PE, bf16): pl[b, lo_j:hi_j] = wtjT_j.T @ S_j with S_j a
    0/1 band matrix; disjoint bands + fixup columns for straddlers.
  * Drain pl (PSUM f32) to SBUF split across ACT/DVE, DMA out.
"""
import os
import sys
import numpy as np
import ml_dtypes

for _p in ("/opt/trn_rl_repo",):
    if _p not in sys.path:
        sys.path.append(_p)

import concourse.bass as bass  # noqa: E402
import concourse.tile as tile  # noqa: E402
from concourse import bacc, mybir, bass_utils  # noqa: E402
from contextlib import ExitStack  # noqa: E402

F32 = mybir.dt.float32
BF16 = mybir.dt.bfloat16

B, F, G, L, C = 32768, 512, 16, 64, 1000
NCORE = 8
BC = B // NCORE          # 4096 batch rows per core
NT = BC // 512           # 8 b-tiles of 512
KF = F // 128            # 4 feature chunks
GL = G * L               # 1024 heads
NCH = GL // 128          # 8 head chunks, no padding

LAST_EXEC_NS = None


def _ensure_ntff_hook():
    """Some images ship an `antenv` without the optional `axon_hooks`
    submodule; bass_utils then crashes on import when tracing. Provide
    the module and register the ctypes NTFF hook trn_boot would have."""
    try:
        from antenv import axon_hooks  # noqa: F401
        return
    except ImportError:
        pass
    import types
    import antenv
    mod = types.ModuleType("antenv.axon_hooks")
    _hook = [None]
    mod.set_axon_ntff_profile_hook = lambda h: _hook.__setitem__(0, h)
    mod.get_axon_ntff_profile_hook = lambda: _hook[0]
    sys.modules["antenv.axon_hooks"] = mod
    antenv.axon_hooks = mod
    try:
        from trn_agent_boot.trn_boot import _ntff_profile_via_ctypes
        h = _ntff_profile_via_ctypes("/opt/axon/libaxon_pjrt.so")
        if h is not None:
            mod.set_axon_ntff_profile_hook(h)
    except Exception:
        pass


def _host_prep(W, b, label_ids):
    lab = np.asarray(label_ids).reshape(-1).astype(np.int64)
    order = np.argsort(lab, kind="stable")
    lab_s = lab[order]                      # ascending classes, len 1024

    # bands + straddle fixups
    bands = []
    fixups = []                             # (chunk_j, cstar, fix_col)
    lo = 0
    for j in range(NCH):
        if j < NCH - 1:
            c_end = int(lab_s[128 * (j + 1) - 1])
            c_next = int(lab_s[128 * (j + 1)])
            if c_end == c_next:
                hi = c_end + 1
                fixups.append((j + 1, c_end, len(fixups)))
            else:
                hi = c_next
        else:
            hi = C
        bands.append((lo, hi))
        lo = hi

    straddle_tail = {(j, c) for (j, c, _) in fixups}
    NFIX = max(len(fixups), 1)
    S_cat = np.zeros((128, C), dtype=ml_dtypes.bfloat16)
    S_fix = np.zeros((128, NFIX), dtype=ml_dtypes.bfloat16)
    fix_of = {(j, c): col for (j, c, col) in fixups}
    for p in range(GL):
        j, r = p // 128, p % 128
        c = int(lab_s[p])
        if (j, c) in straddle_tail:
            S_fix[r, fix_of[(j, c)]] = 1.0
        else:
            S_cat[r, c] = 1.0

    Wflat = np.asarray(W).reshape(GL, F).astype(np.float32)
    bflat = np.asarray(b).reshape(GL).astype(np.float32)
    Wsorted = Wflat[order]                  # [1024, F]
    # wt tiles: [KF, 2, 128, 512] -> contiguous [128,512] blocks
    WT2 = np.ascontiguousarray(
        Wsorted.T.reshape(KF, 128, GL).reshape(KF, 128, 2, 512)
        .transpose(0, 2, 1, 3)).astype(ml_dtypes.bfloat16)
    biasT = np.zeros((128, NCH), dtype=np.float32)
    E = np.zeros((16, GL), dtype=ml_dtypes.bfloat16)
    for p, gl in enumerate(order):
        biasT[p % 128, p // 128] = bflat[gl]
        E[gl // L, p] = 1.0
    return dict(bands=bands, fixups=fixups, NFIX=NFIX, S_cat=S_cat,
                S_fix=S_fix, WT2=WT2, biasT=biasT, E=E)


def _band_segments(lo, hi, fix_cols):
    """Split [lo, hi) at 512-col (PSUM bank) boundaries; a segment whose
    range contains a fixup target column keeps its accumulation open."""
    segs = []
    while lo < hi:
        nxt = min(hi, (lo // 512 + 1) * 512)
        stop = not any(lo <= c < nxt for c in fix_cols)
        segs.append((lo, nxt, stop))
        lo = nxt
    return segs


def _build_program(bands, fixups, NFIX):
    nc = bacc.Bacc("TRN2", target_bir_lowering=False, debug=False,
                   num_devices=NCORE)
    xt_d = nc.dram_tensor("xt", [KF * NT * 128, 512], BF16,
                          kind="ExternalInput").ap()
    pt_d = nc.dram_tensor("pt", [16, BC], BF16, kind="ExternalInput").ap()
    wt_d = nc.dram_tensor("wt", [KF * 2 * 128, 512], BF16,
                          kind="ExternalInput").ap()
    e_d = nc.dram_tensor("e", [16, GL], BF16, kind="ExternalInput").ap()
    bt_d = nc.dram_tensor("bt", [128, NCH], F32, kind="ExternalInput").ap()
    s_d = nc.dram_tensor("s", [128, C], BF16, kind="ExternalInput").ap()
    sf_d = nc.dram_tensor("sf", [128, NFIX], BF16, kind="ExternalInput").ap()
    out_d = nc.dram_tensor("logits", [BC, C], F32, kind="ExternalOutput").ap()

    # fixup targets per chunk: chunk j -> [(cstar, col)]
    fix_by_chunk = {}
    for (j, c, col) in fixups:
        fix_by_chunk.setdefault(j, []).append((c, col))
    # per chunk: columns that a LATER fixup will accumulate into
    fix_cols_of_band = {}
    for (j, c, col) in fixups:
        fix_cols_of_band.setdefault(j - 1, []).append(c)

    with tile.TileContext(nc) as tc, ExitStack() as ctx:
        const = ctx.enter_context(tc.tile_pool(name="const", bufs=1))
        psG = ctx.enter_context(tc.tile_pool(name="psG", bufs=2, space="PSUM"))
        psM = ctx.enter_context(tc.tile_pool(name="psM", bufs=3, space="PSUM"))
        psLA = ctx.enter_context(tc.tile_pool(name="psLA", bufs=2, space="PSUM"))
        psLB = ctx.enter_context(tc.tile_pool(name="psLB", bufs=1, space="PSUM"))
        sbW = ctx.enter_context(tc.tile_pool(name="sbW", bufs=18))
        sbP = ctx.enter_context(tc.tile_pool(name="sbP", bufs=4))
        sbO = ctx.enter_context(tc.tile_pool(name="sbO", bufs=6))

        # small consts on the gpsimd (software) queue; pts first (pm j=0 dep)
        pts = const.tile([16, BC], BF16, name="pts", tag="pts")
        nc.gpsimd.dma_start(pts[:], pt_d[:])
        es = const.tile([16, GL], BF16, name="es", tag="es")
        nc.gpsimd.dma_start(es[:], e_d[:])
        bts = const.tile([128, NCH], F32, name="bts", tag="bts")
        nc.gpsimd.dma_start(bts[:], bt_d[:])
        ss = const.tile([128, C], BF16, name="ss", tag="ss")
        nc.gpsimd.dma_start(ss[:], s_d[:])
        sfs = const.tile([128, NFIX], BF16, name="sfs", tag="sfs")
        nc.gpsimd.dma_start(sfs[:], sf_d[:])

        # weights + activations on the sync hw queue; first-gemm deps first
        wts = [[None, None] for _ in range(KF)]
        xts = [[None] * NT for _ in range(KF)]

        def load_wt(k, h):
            t_ = const.tile([128, 512], BF16, name=f"w{k}_{h}", tag=f"w{k}_{h}")
            nc.sync.dma_start(t_[:], wt_d[(k * 2 + h) * 128:
                                          (k * 2 + h + 1) * 128, :])
            wts[k][h] = t_

        def load_x(k, t, eng):
            t_ = const.tile([128, 512], BF16, name=f"x{k}_{t}", tag=f"x{k}_{t}")
            eng.dma_start(t_[:], xt_d[(k * NT + t) * 128:
                                      (k * NT + t + 1) * 128, :])
            xts[k][t] = t_

        # first-tile x on the scalar queue so its transfers run in
        # parallel with the wt loads on sync during startup
        for k in range(KF):
            load_wt(k, 0)
            load_x(k, 0, nc.scalar)
        for k in range(KF):
            load_wt(k, 1)
        for t in range(1, NT):
            for k in range(KF):
                load_x(k, t, nc.sync)

        def wt_slice(k, j):
            """lhsT [128, 128] for feature-chunk k, head-chunk j."""
            h, off = divmod(j * 128, 512)
            return wts[k][h][:, off:off + 128]

        # tiles: 7 full 512-row tiles + 2 half tiles at the end, so the
        # final scatter tail only covers 256 rows
        TILES = [(ti * 512, 512) for ti in range(NT - 1)]
        TILES += [(3584, 256), (3840, 256)]

        all_wtjs = {}

        def gemm_chunk(ti, j):
            row0, w = TILES[ti]
            tx, off = divmod(row0, 512)
            pg = psG.tile([128, 512], F32, name="pg", tag="pg")
            pm = psM.tile([128, 512], F32, name="pm", tag="pm")
            for k in range(KF):
                nc.tensor.matmul(pg[:, :w], wt_slice(k, j),
                                 xts[k][tx][:, off:off + w],
                                 start=(k == 0), stop=(k == KF - 1))
                if k == 1:
                    # interleave the prob-gather matmul inside the open pg
                    # accumulation group: LDWEIGHTS after an in-group matmul
                    # overlaps its streaming; after a group close it stalls
                    # until the array drains (~300ns per transition)
                    nc.tensor.matmul(pm[:, :w], es[:, bass.ts(j, 128)],
                                     pts[:, row0:row0 + w],
                                     start=True, stop=True)
            # hw limit: only one PSUM operand per DVE op -> drain pg on ACT
            gb = sbP.tile([128, 512], BF16, name="gb", tag="gb")
            nc.scalar.activation(gb[:, :w], pg[:, :w],
                                 mybir.ActivationFunctionType.Identity,
                                 bias=bts[:, j:j + 1], scale=1.0)
            wtj = sbW.tile([128, 512], BF16, name="wtj", tag="wtj")
            nc.vector.tensor_mul(wtj[:, :w], gb[:, :w], pm[:, :w])
            all_wtjs.setdefault(ti, []).append(wtj)

        pending_drain = []

        def scatter_mm(ti, bs_i):
            wtjs = all_wtjs[ti]
            bsl = bass.ts(bs_i, 128)
            # two single-bank tiles so each drains/recycles independently
            plA = psLA.tile([128, 512], F32, name="plA", tag="plA")
            plB = psLB.tile([128, 512], F32, name="plB", tag="plB")

            def pslice(n0, n1):
                return plA[:, n0:n1] if n1 <= 512 else plB[:, n0 - 512:n1 - 512]

            for j, (lo, hi) in enumerate(bands):
                for (n0, n1, stop) in _band_segments(
                        lo, hi, fix_cols_of_band.get(j, ())):
                    nc.tensor.matmul(pslice(n0, n1), wtjs[j][:, bsl],
                                     ss[:, n0:n1], start=True, stop=stop)
                # a later band's start=True re-marks the whole 2KB zero
                # region pending-zero, which would wipe the straddle column
                # -> the fixup accumulate must run before band j+1 starts
                for (cstar, col) in fix_by_chunk.get(j + 1, ()):
                    nc.tensor.matmul(pslice(cstar, cstar + 1),
                                     wtjs[j + 1][:, bsl],
                                     sfs[:, col:col + 1],
                                     start=False, stop=True)
            pending_drain.append((ti, bs_i, plA, plB))

        def scatter_drain():
            while pending_drain:
                ti, bs_i, plA, plB = pending_drain.pop(0)
                row0 = TILES[ti][0] + bs_i * 128
                ob = sbO.tile([128, C], F32, name="ob", tag="ob")
                # gpsimd cannot read PSUM; split the drain ACT/DVE
                nc.scalar.activation(ob[:, :512], plA[:],
                                     mybir.ActivationFunctionType.Identity,
                                     bias=0.0, scale=1.0)
                nc.vector.tensor_copy(ob[:, 512:C], plB[:, 0:C - 512])
                # alternate output queues: keep half off the input (sync)
                # queue and half off the scalar engine's stream
                eng = nc.scalar if bs_i % 2 == 0 else nc.sync
                eng.dma_start(out_d[row0:row0 + 128, :], ob[:])

        # software pipeline: scatter(t-1) subtiles interleaved into gemm(t);
        # drains emitted one chunk later so gb/mult stay ahead in the
        # ACT/DVE queues
        NTI = len(TILES)
        for ti in range(NTI):
            nsub_prev = TILES[ti - 1][1] // 128 if ti > 0 else 0
            slots = {3: 0, 7: 1} if nsub_prev == 2 else \
                    {1: 0, 3: 1, 5: 2, 7: 3}
            for j in range(NCH):
                gemm_chunk(ti, j)
                scatter_drain()
                if ti > 0 and j in slots:
                    scatter_mm(ti - 1, slots[j])
            if ti > 0:
                all_wtjs.pop(ti - 1)
        for bs_i in range(TILES[-1][1] // 128):
            scatter_mm(NTI - 1, bs_i)
            scatter_drain()
        all_wtjs.pop(NTI - 1)
    nc.finalize()
    return nc


def kernel(features, group_probs, W, b, label_ids):
    global LAST_EXEC_NS
    features = np.asarray(features, dtype=np.float32)
    group_probs = np.asarray(group_probs, dtype=np.float32)
    prep = _host_prep(W, b, label_ids)
    nc = _build_program(prep["bands"], prep["fixups"], prep["NFIX"])

    Xb = features.astype(ml_dtypes.bfloat16)
    PT = np.ascontiguousarray(group_probs.T.astype(ml_dtypes.bfloat16))
    WT2 = np.ascontiguousarray(prep["WT2"].reshape(KF * 2 * 128, 512))
    in_maps = []
    for c in range(NCORE):
        Xc = Xb[c * BC:(c + 1) * BC]                      # [4096, 512]
        # [KF, NT, 128, 512]: tile (k, t) = Xc[t*512:(t+1)*512,
        #                                      k*128:(k+1)*128].T
        XT2 = np.ascontiguousarray(
            Xc.reshape(NT, 512, KF, 128).transpose(2, 0, 3, 1))
        in_maps.append({
            "xt": XT2.reshape(KF * NT * 128, 512),
            "pt": np.ascontiguousarray(PT[:, c * BC:(c + 1) * BC]),
            "wt": WT2,
            "e": prep["E"],
            "bt": prep["biasT"],
            "s": prep["S_cat"],
            "sf": prep["S_fix"],
        })

    trace = bool(os.environ.get("BASS_TRACE"))
    if trace:
        bass_utils.upload_artifacts = lambda d: "local://skipped"
        _ensure_ntff_hook()
    try:
        res = bass_utils.run_bass_kernel_spmd(nc, in_maps,
                                              core_ids=list(range(NCORE)))
    except Exception:
        # transient NRT device errors have been observed; one retry
        res = bass_utils.run_bass_kernel_spmd(nc, in_maps,
                                              core_ids=list(range(NCORE)))
    if trace:
        LAST_EXEC_NS = res.exec_time_ns
        if res.exec_time_ns is not None:
            print(f"HW exec time: {res.exec_time_ns} ns")

    out = np.concatenate([res.results[c]["logits"] for c in range(NCORE)],
                         axis=0)
    return np.ascontiguousarray(out.astype(np.float32))
